# revision 50
# baseline (speedup 1.0000x reference)
"""Causal linear attention (elu+1 feature map) on 8 trn2 NeuronCores.

Sharding: core = 4*b + g  (b = batch 0..1, g = head-group 0..3, 4 heads each).
Per core: fp16 qkv projection for its 4 heads (w_attn column shard), chunked
causal linear attention (DxD state recurrence in PSUM), row-sharded output
projection giving a partial fp16 (T, C) output. Host sums the 4 head-group
partials per batch in fp32.

Layouts on chip (per core), heads h = 2*j + hp at partition base hp*64:
  xt   (C=1024, T=2048) fp16    - x[b] transposed on host
  q2,k2 (128, 2, T) fp16        - feature-major phi(qkv); pair j, head-half hp
  v_all (128, 16, 4, 65) fp16   - token-major per chunk, col 64 = ones
  attention chunk L=128, all 4 heads batched per PSUM bank:
    at4 (128, 4*128) = K^T Q per head -> mask -> atm4 fp16
    y_tok4 (128, 4, 65) = A V' + Q^T S'   (token-major, col 64 = denominator)
    s2[j] (128, 65) PSUM accumulates S' = [S | z] for heads (2j, 2j+1)
    division: per-partition reciprocal of col 64, tensor_scalar per head
    ydT = PE-transpose of ydiv pairs -> feature-major fp16 for projection
  proj: out(t, c) = ydT(2 pair tiles).T @ w_proj_shard, fp16 out DMA
"""

import numpy as np

import concourse.bass as bass
import concourse.mybir as mybir
import concourse.tile as tile
from concourse import bacc
from concourse.bass_utils import run_bass_kernel_spmd

F32 = mybir.dt.float32
F16 = mybir.dt.float16
AF = mybir.ActivationFunctionType
ALU = mybir.AluOpType

B, T, C = 2, 2048, 1024
H, D = 16, 64
NCORES = 8
HL = 4  # heads per core
FQK = HL * D  # 256 local features for each of q, k, v
L = 128  # attention chunk length
NCH = T // L  # 16 chunks
KT = C // 128  # 8 contraction tiles


def build_nc():
    nc = bacc.Bacc("TRN2", target_bir_lowering=False, debug=False, num_devices=NCORES)

    xt_d = nc.dram_tensor("xt", (C, T), F16, kind="ExternalInput")
    ws_d = nc.dram_tensor("ws", (C, 3 * FQK), F16, kind="ExternalInput")
    wp_d = nc.dram_tensor("wp", (FQK, C), F16, kind="ExternalInput")
    mask_d = nc.dram_tensor("mask", (L, 4 * L), F16, kind="ExternalInput")
    id_d = nc.dram_tensor("ident", (128, 192), F16, kind="ExternalInput")
    out_d = nc.dram_tensor("out", (T, C), F16, kind="ExternalOutput")

    xt_ap = xt_d.ap().rearrange("(k p) t -> k p t", p=128)  # (8, 128, 2048)
    ws_ap = ws_d.ap().rearrange("(k p) f -> k p f", p=128)  # (8, 128, 768)
    wp_ap = wp_d.ap().rearrange("(k p) c -> k p c", p=128)  # (2, 128, 1024)

    with tile.TileContext(nc) as tc:
        with (
            tc.tile_pool(name="consts", bufs=1) as consts,
            tc.tile_pool(name="wsp", bufs=1) as wsp,
            tc.tile_pool(name="qk", bufs=1) as qkp,
            tc.tile_pool(name="vp", bufs=1) as vp,
            tc.tile_pool(name="ydp", bufs=1) as ydp,
            tc.tile_pool(name="epi", bufs=3) as epi,
            tc.tile_pool(name="attsb", bufs=3) as attsb,
            tc.tile_pool(name="outp", bufs=3) as outp,
        ):
            # ---- weights + x first (wave A blocks on them), interleaved
            # per contraction tile and split across both HW DGE queues
            # (Sync + Activation) — dispatch is ~600ns serial per queue ----
            ws_t = [wsp.tile([128, 3 * FQK], F16, tag=f"ws{kk}", name=f"ws{kk}") for kk in range(KT)]
            xt_t = [wsp.tile([128, T], F16, tag=f"xt{kk}", name=f"xt{kk}") for kk in range(KT)]
            for kk in range(KT):
                nc.sync.dma_start(out=ws_t[kk][:], in_=ws_ap[kk])
                if kk == 0:
                    # quarters: the first wave-A matmul only needs tb=0
                    for tb in range(4):
                        nc.sync.dma_start(
                            out=xt_t[0][:, tb * 512 : (tb + 1) * 512],
                            in_=xt_ap[0][:, tb * 512 : (tb + 1) * 512],
                        )
                else:
                    nc.sync.dma_start(out=xt_t[kk][:], in_=xt_ap[kk])
            mask_sb = consts.tile([L, 4 * L], F16)
            nc.sync.dma_start(out=mask_sb[:], in_=mask_d.ap()[:])
            id_sb = consts.tile([128, 192], F16)
            nc.sync.dma_start(out=id_sb[:], in_=id_d.ap()[:])
            wp_sb = wsp.tile([128, 2, C], F16)
            for kk in range(2):
                nc.sync.dma_start(out=wp_sb[:, kk, :], in_=wp_ap[kk])

            # ---- persistent activations ----
            q2_sb = qkp.tile([128, 2, T], F16)
            k2_sb = qkp.tile([128, 2, T], F16)
            v_all = vp.tile([128, NCH, HL, D + 1], F16)
            ydT_all = ydp.tile([128, NCH, 2, L], F16)

            if True:
                nc.vector.memset(v_all[:, :, :, D : D + 1], 1.0)

                def phi_epi(ps, dst):
                    # phi = min(exp(x),1) + relu(x), full 128-partition width
                    e_t = epi.tile([128, 512], F16, tag="e", name="e_t")
                    nc.scalar.activation(out=e_t[:], in_=ps[:], func=AF.Exp)
                    m_t = epi.tile([128, 512], F16, tag="m", name="m_t")
                    nc.vector.tensor_scalar_min(m_t[:], e_t[:], 1.0)
                    nc.vector.scalar_tensor_tensor(
                        out=dst,
                        in0=ps[:],
                        scalar=0.0,
                        in1=m_t[:],
                        op0=ALU.max,
                        op1=ALU.add,
                    )

                with tc.tile_pool(name="mm", bufs=1, space="PSUM") as mmp:
                    # ---- wave A (q): kk-outer so matmuls chase the input
                    # DMAs; last kk round goes tile-by-tile so the phi
                    # epilogues stagger instead of bunching at the end ----
                    ps_t = [
                        mmp.tile([128, 512], F32, tag=f"t{ix}", name=f"ps{ix}")
                        for ix in range(8)
                    ]
                    for kk in range(KT - 1):
                        for ix in range(8):
                            fo, tb = ix // 4, ix % 4
                            nc.tensor.matmul(
                                ps_t[ix][:],
                                ws_t[kk][:, fo * 128 : (fo + 1) * 128],
                                xt_t[kk][:, tb * 512 : (tb + 1) * 512],
                                start=(kk == 0),
                                stop=False,
                            )
                    for ix in range(8):
                        fo, tb = ix // 4, ix % 4
                        nc.tensor.matmul(
                            ps_t[ix][:],
                            ws_t[KT - 1][:, fo * 128 : (fo + 1) * 128],
                            xt_t[KT - 1][:, tb * 512 : (tb + 1) * 512],
                            start=False,
                            stop=True,
                        )
                        phi_epi(ps_t[ix], q2_sb[:, fo, tb * 512 : (tb + 1) * 512])

                    # ---- wave B (k): tile-outer, epilogue pipelined ----
                    for ix in range(8):
                        fo, tb = ix // 4, ix % 4
                        ps = mmp.tile([128, 512], F32, tag=f"t{ix}", name=f"psb{ix}")
                        for kk in range(KT):
                            nc.tensor.matmul(
                                ps[:],
                                ws_t[kk][:, (2 + fo) * 128 : (3 + fo) * 128],
                                xt_t[kk][:, tb * 512 : (tb + 1) * 512],
                                start=(kk == 0),
                                stop=(kk == KT - 1),
                            )
                        phi_epi(ps, k2_sb[:, fo, tb * 512 : (tb + 1) * 512])

                    # ---- waves C, D: v token-major, tile-outer ----
                    for tt in range(NCH):
                        ps = mmp.tile(
                            [128, FQK], F32, tag=f"t{tt % 8}", name=f"psv{tt}"
                        )
                        for kk in range(KT):
                            nc.tensor.matmul(
                                ps[:],
                                xt_t[kk][:, tt * 128 : (tt + 1) * 128],
                                ws_t[kk][:, 2 * FQK : 3 * FQK],
                                start=(kk == 0),
                                stop=(kk == KT - 1),
                            )
                        nc.scalar.copy(
                            out=v_all[:, tt, :, 0:D],
                            in_=ps[:].rearrange("p (h d) -> p h d", d=D),
                        )

            # ---- attention + inlined projection ----
            with tc.tile_pool(name="atps", bufs=1, space="PSUM") as atps, \
                 tc.tile_pool(name="trps", bufs=2, space="PSUM") as trps, \
                 tc.tile_pool(name="ytps", bufs=2, space="PSUM") as ytps, \
                 tc.tile_pool(name="sps", bufs=1, space="PSUM") as sps:
                # bank j holds S' for heads (2j, 2j+1) side by side in columns
                s2 = [
                    sps.tile([D, 2, D + 1], F32, tag=f"s{j}", name=f"s{j}")
                    for j in range(2)
                ]
                # S' staging, zero-padded to 128 contraction rows; ping-pong
                # pair memset once, zero halves never rewritten
                ssb_pp = [
                    qkp.tile([128, HL, D + 1], F16, tag=f"sspp{x}", name=f"sspp{x}")
                    for x in range(2)
                ]
                for x in range(2):
                    nc.vector.memset(ssb_pp[x][:], 0.0)

                def emit_ydT(pi, pydv):
                    # transpose head-pairs to feature-major for projection
                    for j in range(2):
                        ydT = trps.tile([L, L], F16, tag="tp", name="ydT")
                        nc.tensor.matmul(
                            ydT[:],
                            pydv[:, 2 * j : 2 * j + 2, :].rearrange("p a b -> p (a b)"),
                            id_sb[:, 64:192],
                            is_transpose=True,
                            start=True,
                            stop=True,
                        )
                        nc.scalar.copy(out=ydT_all[:, pi, j, :], in_=ydT[:])

                prev_ydv = None
                for i in range(NCH):
                    tsl = slice(i * L, (i + 1) * L)

                    def qk_of(sb, h):
                        base = (h % 2) * 64
                        return sb[base : base + 64, h // 2, tsl]

                    if i > 0:
                        # S' staging sits on the S critical path: issue
                        # first, split across scalar and vector engines.
                        ssb4 = ssb_pp[i % 2]
                        for h in range(HL):
                            m, j = h % 2, h // 2
                            nc.scalar.copy(
                                out=ssb4[m * 64 : m * 64 + 64, h, :],
                                in_=s2[j][:, m, :],
                            )

                    # K^T Q, one PSUM bank per head-parity: all matmuls
                    # grouped into a bank must share one input partition
                    # base (hw crashes on mid-group tile-position change),
                    # and start/stop only once per bank (start marks the
                    # whole bank pending-zero).
                    atm4 = attsb.tile([L, 2, 2 * L], F16, tag="atm4", name="atm4")
                    for m in range(2):
                        at2 = atps.tile(
                            [L, 2 * L], F32, tag=f"at{m}", name=f"at{m}"
                        )
                        for j in range(2):
                            nc.tensor.matmul(
                                at2[:, j * L : (j + 1) * L],
                                qk_of(k2_sb, 2 * j + m),
                                qk_of(q2_sb, 2 * j + m),
                                start=(j == 0),
                                stop=(j == 1),
                            )
                        nc.vector.tensor_mul(
                            atm4[:, m, :], at2[:], mask_sb[:, 0 : 2 * L]
                        )

                    if prev_ydv is not None:
                        emit_ydT(i - 1, prev_ydv)

                    if i < NCH - 1:
                        # transpose k pair-wise: (feat-pair 128, tok 128) ->
                        # (tok, feat-pair); one PSUM bank per transpose (hw
                        # NaNs if transposes share a bank via start/stop)
                        ktok2 = attsb.tile([128, 2, 128], F16, tag="ktok2", name="ktok2")
                        for j in range(2):
                            ktr = trps.tile([128, 128], F16, tag="tp", name="ktr")
                            nc.tensor.matmul(
                                ktr[:],
                                k2_sb[:, j, tsl],
                                id_sb[:, 64:192],
                                is_transpose=True,
                                start=True,
                                stop=True,
                            )
                            nc.scalar.copy(out=ktok2[:, j, :], in_=ktr[:])

                    # y_tok = A V' (+ Q^T S'), token-major, col 64 = denominator
                    yt4 = ytps.tile([L, HL, D + 1], F32, tag="ytok", name="yt4")
                    for h in range(HL):
                        nc.tensor.matmul(
                            yt4[:, h, :],
                            atm4[:, h % 2, (h // 2) * L : (h // 2 + 1) * L],
                            v_all[:, i, h, :],
                            start=(h == 0),
                            stop=(i == 0 and h == HL - 1),
                        )
                    if i > 0:
                        # heads 2j, 2j+1 share the q-pair lhsT and their yt4
                        # columns are adjacent: one matmul per pair
                        for j in range(2):
                            nc.tensor.matmul(
                                yt4[:, 2 * j : 2 * j + 2, :],
                                q2_sb[:, j, tsl],
                                ssb4[:, 2 * j : 2 * j + 2, :],
                                start=False,
                                stop=(j == 1),
                            )

                    if i < NCH - 1:
                        for h in range(HL):
                            m, j = h % 2, h // 2
                            nc.tensor.matmul(
                                s2[j][:, m, :],
                                ktok2[:, j, m * 64 : m * 64 + 64],
                                v_all[:, i, h, :],
                                start=(i == 0 and m == 0),
                                stop=(i == NCH - 2 and m == 1),
                                # group stays open across chunks while ssb4
                                # copies read the partial S (fine on hw)
                                skip_group_check=True,
                            )

                    # division by denominator, still token-major; all muls on
                    # the vector engine (scalar is the busier engine here)
                    r4 = attsb.tile([L, HL], F32, tag="r4", name="r4")
                    nc.vector.reciprocal_approx_fast(out=r4[:], in_=yt4[:, :, D])
                    ydv4 = attsb.tile([L, HL, D], F16, tag="ydv4", name="ydv4")
                    for h in range(HL):
                        with nc.allow_low_precision(reason="y to fp16"):
                            nc.vector.tensor_scalar_mul(
                                ydv4[:, h, :], yt4[:, h, 0:D], r4[:, h : h + 1]
                            )

                    prev_ydv = ydv4
                emit_ydT(NCH - 1, prev_ydv)

            # ---- output projection ----
            with tc.tile_pool(name="pps", bufs=4, space="PSUM") as pps:
                for tt in range(NCH):
                    tpsl = slice(tt * 128, (tt + 1) * 128)
                    po = [None, None]
                    for cb in range(2):
                        ps = pps.tile([128, 512], F32, tag="po", name="po")
                        for hp in range(2):
                            nc.tensor.matmul(
                                ps[:],
                                ydT_all[:, tt, hp, :],
                                wp_sb[:, hp, cb * 512 : (cb + 1) * 512],
                                start=(hp == 0),
                                stop=(hp == 1),
                            )
                        po[cb] = ps
                    os_t = outp.tile([128, C], F16, tag="os", name="os_t")
                    # copies split across scalar and vector: scalar alone
                    # (21.8us for 32 copies) outruns the PE and leaves a
                    # 12us serial drain after the last matmul
                    with nc.allow_low_precision(reason="out to fp16"):
                        nc.scalar.copy(out=os_t[:, 0:512], in_=po[0][:])
                        nc.vector.tensor_copy(out=os_t[:, 512:1024], in_=po[1][:])
                    nc.sync.dma_start(out=out_d.ap()[tpsl, :], in_=os_t[:])

    nc.compile()
    return nc


_NC = None


def _get_nc():
    global _NC
    if _NC is None:
        _NC = build_nc()
    return _NC


def make_in_maps(x, w_attn, w_proj):
    tri = np.triu(np.ones((L, L), dtype=np.float16))
    mask = np.tile(tri, (1, HL))  # (128, 512)
    ident = np.zeros((128, 192), dtype=np.float16)
    i64 = np.eye(64, dtype=np.float16)
    ident[0:64, 0:64] = i64
    ident[64:128, 0:64] = i64
    ident[:, 64:192] = np.eye(128, dtype=np.float16)
    in_maps = []
    for core in range(NCORES):
        b, g = core // HL, core % HL
        cols = slice(g * FQK, (g + 1) * FQK)
        xt = np.ascontiguousarray(x[b].T).astype(np.float16)
        ws = np.ascontiguousarray(
            np.concatenate(
                [w_attn[:, 0 * C :][:, cols], w_attn[:, C : 2 * C][:, cols],
                 w_attn[:, 2 * C :][:, cols]],
                axis=1,
            )
        ).astype(np.float16)
        wp = np.ascontiguousarray(w_proj[g * FQK : (g + 1) * FQK, :]).astype(
            np.float16
        )
        in_maps.append(dict(xt=xt, ws=ws, wp=wp, mask=mask, ident=ident))
    return in_maps


def kernel(x, w_attn, w_proj):
    x = np.asarray(x)
    w_attn = np.asarray(w_attn)
    w_proj = np.asarray(w_proj)
    nc = _get_nc()
    res = run_bass_kernel_spmd(
        nc, make_in_maps(x, w_attn, w_proj), core_ids=list(range(NCORES))
    )
    out = np.zeros((B, T, C), dtype=np.float32)
    for core in range(NCORES):
        out[core // HL] += res.results[core]["out"].astype(np.float32)
    return out


if __name__ == "__main__":
    rng = np.random.default_rng(0)
    x = rng.standard_normal((B, T, C)).astype(np.float32)
    wa = (rng.standard_normal((C, 3 * C)) * 0.02).astype(np.float32)
    wp = (rng.standard_normal((C, C)) * 0.02).astype(np.float32)
    o = kernel(x, wa, wp)
    print("out", o.shape, o.dtype, float(np.abs(o).max()))
